# revision 7
# baseline (speedup 1.0000x reference)
"""Trainium2 Bass kernel for CompactCenterLoss (B=4096, D=512, C=100, 8 cores).

Math notes (vs the reference):
  dist[i, j] = ||x_i - centers[t_j]|| depends on j only through the class
  t_j, so the first BxB table collapses to a [B, C] table D2:
      dist_ap[i] = D2[i, t_i]                      (all same-class j equal)
      dist_an[i] = min_{c present, c != t_i} D2[i, c]
  Only pdist(x, x) needs the full BxB compute. Its masked row sums are
  obtained per class via a matmul with the one-hot matrix O [B, C]:
      S[i, c]   = sum_j dist[i, j] * O[j, c]
      pos_sum   = S[i, t_i],  tot_sum = sum_c S[i, c]  (rows of O sum to 1)

Sharding: batch rows are split across 8 cores (512 rows each). Every core
computes dist^T tiles [128 j x 512 i_shard] against the full (replicated)
input, using a per-core ROTATION of the j axis so the diagonal block is
always device-j-tiles 0..3 -- a single uniform SPMD program for all cores.
The diagonal is forced to exact zero (reference's clipped diagonal is
~1e-6, negligible); sqrt inputs are kept positive by adding +2*BIG to the
diagonal before the sqrt and multiplying by (1 - I) after it.

Precision: the big Gram matmul runs in bf16 (full PE rate; error on loss2
~1e-4 relative), the small [B, C] table in fp32 (loss1/prec decisions).
"""

import numpy as np
import ml_dtypes
from contextlib import ExitStack

import jax
import concourse.bass as bass
import concourse.tile as tile
import concourse.mybir as mybir
from concourse import bacc
from concourse.bass2jax import install_neuronx_cc_hook, _bass_exec_p, partition_id_tensor

B, D, C = 4096, 512, 100
N_CORES = 8
P = 128
SH = B // N_CORES          # 512 rows per core
NJ = B // P                # 32 j-tiles
KT = D // P                # 4 k-tiles
NCH = SH // P              # 4 i-chunks per core
CCW = 1024                 # xt column-chunk width (for DMA/compute overlap)
NCC = B // CCW             # 4 column chunks
BIG = 1.0e12
IAML_MARGIN = 5.0

f32 = mybir.dt.float32
bf16 = mybir.dt.bfloat16
BF16_NP = ml_dtypes.bfloat16

Alu = mybir.AluOpType
Act = mybir.ActivationFunctionType


def _build_program():
    nc = bacc.Bacc("TRN2", target_bir_lowering=False, debug=False,
                   enable_asserts=True, num_devices=1)

    # ---- DRAM I/O (per core; host pre-rotates the j axis by the shard offset)
    xt_d = nc.dram_tensor("xt", [D, B], bf16, kind="ExternalInput")       # X^T, cols rotated
    xst_d = nc.dram_tensor("xst", [D, SH], f32, kind="ExternalInput")     # X_shard^T (fp32)
    ct_d = nc.dram_tensor("ct", [D, C], f32, kind="ExternalInput")        # centers^T
    ohp_d = nc.dram_tensor("ohp", [P, NJ * C], bf16, kind="ExternalInput")  # one-hot, j-tiled+rotated
    ohs_d = nc.dram_tensor("ohs", [SH, C], f32, kind="ExternalInput")     # one-hot shard rows
    ohsb_d = nc.dram_tensor("ohsb", [SH, C], f32, kind="ExternalInput")   # BIG * one-hot shard rows
    sqxib_d = nc.dram_tensor("sqxib", [P, SH], f32, kind="ExternalInput")  # |x_i|^2 bcast over partitions
    sqxc_d = nc.dram_tensor("sqxc", [P, NJ], f32, kind="ExternalInput")   # |x_j|^2 per j-tile col (rotated)
    sqcb_d = nc.dram_tensor("sqcb", [P, C], f32, kind="ExternalInput")    # |c|^2 + BIG*absent, bcast
    sqxs_d = nc.dram_tensor("sqxs", [P, NCH], f32, kind="ExternalInput")  # |x_i|^2 per i-chunk col
    invp_d = nc.dram_tensor("invp", [P, NCH], f32, kind="ExternalInput")  # 1/cnt[t_i]
    invn_d = nc.dram_tensor("invn", [P, NCH], f32, kind="ExternalInput")  # 1/(B-cnt[t_i])
    pI_d = nc.dram_tensor("pI", [P, P], f32, kind="ExternalInput")        # +2*BIG*I
    cI_d = nc.dram_tensor("cI", [P, P], bf16, kind="ExternalInput")       # 1 - I
    out_d = nc.dram_tensor("out", [P, 3 * NCH], f32, kind="ExternalOutput")

    with tile.TileContext(nc) as tc, ExitStack() as ctx:
        singles = ctx.enter_context(tc.tile_pool(name="singles", bufs=1))
        tmp = ctx.enter_context(tc.tile_pool(name="tmp", bufs=2))
        dpool = ctx.enter_context(tc.tile_pool(name="dist", bufs=3))
        spool = ctx.enter_context(tc.tile_pool(name="sq", bufs=3))
        gram_pool = ctx.enter_context(tc.tile_pool(name="gram", bufs=2, space="PSUM"))
        g1_pool = ctx.enter_context(tc.tile_pool(name="g1", bufs=2, space="PSUM"))
        s_pool = ctx.enter_context(tc.tile_pool(name="sacc", bufs=1, space="PSUM"))

        # ---- resident SBUF tiles (small inputs first: phase A starts early)
        xst_sb = []
        for k in range(KT):
            t_ = singles.tile([P, SH], f32, tag=f"xst{k}")
            nc.sync.dma_start(out=t_, in_=xst_d.ap()[k * P:(k + 1) * P, :])
            xst_sb.append(t_)
        ct_sb = []
        for k in range(KT):
            t_ = singles.tile([P, C], f32, tag=f"ct{k}")
            nc.sync.dma_start(out=t_, in_=ct_d.ap()[k * P:(k + 1) * P, :])
            ct_sb.append(t_)
        ohs_sb = []
        ohsb_sb = []
        for k in range(NCH):
            t_ = singles.tile([P, C], f32, tag=f"ohs{k}")
            nc.sync.dma_start(out=t_, in_=ohs_d.ap()[k * P:(k + 1) * P, :])
            ohs_sb.append(t_)
            t_ = singles.tile([P, C], f32, tag=f"ohsb{k}")
            nc.sync.dma_start(out=t_, in_=ohsb_d.ap()[k * P:(k + 1) * P, :])
            ohsb_sb.append(t_)
        sqcb_sb = singles.tile([P, C], f32, tag="sqcb")
        nc.sync.dma_start(out=sqcb_sb, in_=sqcb_d.ap())
        sqxs_sb = singles.tile([P, NCH], f32, tag="sqxs")
        nc.sync.dma_start(out=sqxs_sb, in_=sqxs_d.ap())
        invp_sb = singles.tile([P, NCH], f32, tag="invp")
        nc.sync.dma_start(out=invp_sb, in_=invp_d.ap())
        invn_sb = singles.tile([P, NCH], f32, tag="invn")
        nc.sync.dma_start(out=invn_sb, in_=invn_d.ap())
        sqxib_sb = singles.tile([P, SH], f32, tag="sqxib")
        nc.sync.dma_start(out=sqxib_sb, in_=sqxib_d.ap())
        sqxc_sb = singles.tile([P, NJ], f32, tag="sqxc")
        nc.sync.dma_start(out=sqxc_sb, in_=sqxc_d.ap())
        pI_sb = singles.tile([P, P], f32, tag="pI")
        nc.sync.dma_start(out=pI_sb, in_=pI_d.ap())
        cI_sb = singles.tile([P, P], bf16, tag="cI")
        nc.sync.dma_start(out=cI_sb, in_=cI_d.ap())

        # big inputs: xt in (k-tile x column-chunk) pieces so early j-tiles
        # can start while the tail still streams in
        xt_sb = [[None] * NCC for _ in range(KT)]
        for cc in range(NCC):
            for k in range(KT):
                t_ = singles.tile([P, CCW], bf16, tag=f"xt{k}_{cc}")
                nc.sync.dma_start(
                    out=t_,
                    in_=xt_d.ap()[k * P:(k + 1) * P, cc * CCW:(cc + 1) * CCW])
                xt_sb[k][cc] = t_
        ohp_sb = singles.tile([P, NJ * C], bf16, tag="ohp")
        nc.sync.dma_start(out=ohp_sb, in_=ohp_d.ap())

        out_sb = singles.tile([P, 3 * NCH], f32, tag="out")
        five_sb = singles.tile([P, 1], f32, tag="five")
        nc.vector.memset(five_sb, IAML_MARGIN)

        # ---- phase A: [B, C] center-distance table -> loss1 terms + prec
        for k in range(NCH):
            g1 = g1_pool.tile([P, C], f32)
            for kt in range(KT):
                nc.tensor.matmul(g1, xst_sb[kt][:, k * P:(k + 1) * P], ct_sb[kt],
                                 start=(kt == 0), stop=(kt == KT - 1))
            sa = tmp.tile([P, C], f32, tag="sa")
            # sa = -2*G1 + (|c|^2 + BIG*absent)
            nc.vector.scalar_tensor_tensor(out=sa, in0=g1, scalar=-2.0,
                                           in1=sqcb_sb, op0=Alu.mult, op1=Alu.add)
            d2 = tmp.tile([P, C], f32, tag="d2")
            nc.scalar.activation(out=d2, in_=sa, func=Act.Sqrt,
                                 bias=sqxs_sb[:, k:k + 1], scale=1.0)
            jk = tmp.tile([P, C], f32, tag="jk")
            ap = tmp.tile([P, 1], f32, tag="ap")
            nc.vector.scalar_tensor_tensor(out=jk, in0=d2, scalar=1.0,
                                           in1=ohs_sb[k], op0=Alu.mult,
                                           op1=Alu.mult, accum_out=ap)
            jk2 = tmp.tile([P, C], f32, tag="jk2")
            an = tmp.tile([P, 1], f32, tag="an")
            nc.vector.tensor_add(jk2, d2, ohsb_sb[k])
            nc.vector.tensor_reduce(an, jk2, axis=mybir.AxisListType.X, op=Alu.min)
            diff = tmp.tile([P, 1], f32, tag="diff")
            nc.vector.tensor_sub(diff, ap, an)
            nc.vector.tensor_scalar_max(out_sb[:, 3 * k:3 * k + 1], diff, 0.0)
            nc.vector.tensor_tensor(out=out_sb[:, 3 * k + 1:3 * k + 2],
                                    in0=an, in1=ap, op=Alu.is_gt)

        # ---- phase B: pdist(x, x) tiles + per-class row sums S
        s_acc = [s_pool.tile([P, C], f32, tag=f"s{k}", name=f"s_acc{k}")
                 for k in range(NCH)]
        for t in range(NJ):
            cc, col0 = (t * P) // CCW, (t * P) % CCW
            gram = gram_pool.tile([P, SH], f32)
            for kt in range(KT):
                nc.tensor.matmul(gram, xt_sb[kt][cc][:, col0:col0 + P],
                                 xt_sb[kt][0][:, 0:SH],
                                 start=(kt == 0), stop=(kt == KT - 1))
            # sq = -2*gram + |x_i|^2
            sq = spool.tile([P, SH], f32)
            nc.vector.scalar_tensor_tensor(out=sq, in0=gram, scalar=-2.0,
                                           in1=sqxib_sb, op0=Alu.mult, op1=Alu.add)
            if t < NCH:
                # diagonal block: push sqrt input to +2*BIG (positive)
                nc.vector.tensor_tensor(out=sq[:, t * P:(t + 1) * P],
                                        in0=sq[:, t * P:(t + 1) * P],
                                        in1=pI_sb, op=Alu.add)
            dist = dpool.tile([P, SH], bf16)
            nc.scalar.activation(out=dist, in_=sq, func=Act.Sqrt,
                                 bias=sqxc_sb[:, t:t + 1], scale=1.0)
            if t < NCH:
                # zero the diagonal exactly
                nc.vector.tensor_tensor(out=dist[:, t * P:(t + 1) * P],
                                        in0=dist[:, t * P:(t + 1) * P],
                                        in1=cI_sb, op=Alu.mult)
            for k in range(NCH):
                nc.tensor.matmul(s_acc[k], dist[:, k * P:(k + 1) * P],
                                 ohp_sb[:, t * C:(t + 1) * C],
                                 start=(t == 0), stop=(t == NJ - 1),
                                 skip_group_check=True)

        # ---- tail: loss2 terms from S
        for k in range(NCH):
            jk3 = tmp.tile([P, C], f32, tag="jk3")
            pos = tmp.tile([P, 1], f32, tag="pos")
            nc.vector.scalar_tensor_tensor(out=jk3, in0=s_acc[k], scalar=1.0,
                                           in1=ohs_sb[k], op0=Alu.mult,
                                           op1=Alu.mult, accum_out=pos)
            tot = tmp.tile([P, 1], f32, tag="tot")
            nc.vector.reduce_sum(tot, s_acc[k], axis=mybir.AxisListType.X)
            neg = tmp.tile([P, 1], f32, tag="neg")
            nc.vector.tensor_sub(neg, tot, pos)
            posm = tmp.tile([P, 1], f32, tag="posm")
            nc.vector.tensor_mul(posm, pos, invp_sb[:, k:k + 1])
            negm = tmp.tile([P, 1], f32, tag="negm")
            nc.vector.tensor_mul(negm, neg, invn_sb[:, k:k + 1])
            r = tmp.tile([P, 1], f32, tag="r")
            nc.scalar.activation(out=r, in_=negm, func=Act.Relu,
                                 bias=five_sb[:, 0:1], scale=-1.0)
            nc.vector.tensor_add(out_sb[:, 3 * k + 2:3 * k + 3], posm, r)

        nc.sync.dma_start(out=out_d.ap(), in_=out_sb)

    nc.compile()
    return nc


_RUNNER = None


def _make_runner():
    """Build the program once and return a cached callable
    in_maps -> list of per-core {"out": np.ndarray}. Mirrors
    concourse.bass2jax.run_bass_via_pjrt but keeps the jitted executable
    alive so repeated kernel() calls don't recompile."""
    import jax.numpy as jnp
    from jax.sharding import Mesh, PartitionSpec
    from jax.experimental.shard_map import shard_map

    nc = _build_program()
    install_neuronx_cc_hook()

    partition_name = nc.partition_id_tensor.name if nc.partition_id_tensor else None
    in_names, out_names, out_avals, zero_shapes = [], [], [], []
    for alloc in nc.m.functions[0].allocations:
        if not isinstance(alloc, mybir.MemoryLocationSet):
            continue
        name = alloc.memorylocations[0].name
        if alloc.kind == "ExternalInput":
            if name != partition_name:
                in_names.append(name)
        elif alloc.kind == "ExternalOutput":
            shape = tuple(alloc.tensor_shape)
            dtype = mybir.dt.np(alloc.dtype)
            out_names.append(name)
            out_avals.append(jax.core.ShapedArray(shape, dtype))
            zero_shapes.append((shape, dtype))
    n_params = len(in_names)
    n_outs = len(out_avals)
    all_in_names = list(in_names) + list(out_names)
    if partition_name is not None:
        all_in_names.append(partition_name)
    donate = tuple(range(n_params, n_params + n_outs))

    def _body(*args):
        operands = list(args)
        if partition_name is not None:
            operands.append(partition_id_tensor())
        outs = _bass_exec_p.bind(
            *operands,
            out_avals=tuple(out_avals),
            in_names=tuple(all_in_names),
            out_names=tuple(out_names),
            lowering_input_output_aliases=(),
            sim_require_finite=True,
            sim_require_nnan=True,
            nc=nc,
        )
        return tuple(outs)

    devices = jax.devices()[:N_CORES]
    mesh = Mesh(np.asarray(devices), ("core",))
    in_specs = (PartitionSpec("core"),) * (n_params + n_outs)
    out_specs = (PartitionSpec("core"),) * n_outs
    sharded = jax.jit(
        shard_map(_body, mesh=mesh, in_specs=in_specs, out_specs=out_specs,
                  check_rep=False),
        donate_argnums=donate, keep_unused=True)

    def run(in_maps):
        concat_in = [
            np.concatenate([np.asarray(in_maps[c][name]) for c in range(N_CORES)],
                           axis=0)
            for name in in_names
        ]
        concat_zeros = [np.zeros((N_CORES * s[0], *s[1:]), dt)
                        for (s, dt) in zero_shapes]
        out_arrs = sharded(*concat_in, *concat_zeros)
        return [
            {name: np.asarray(out_arrs[i]).reshape(N_CORES, *out_avals[i].shape)[c]
             for i, name in enumerate(out_names)}
            for c in range(N_CORES)
        ]

    return run


def _get_runner():
    global _RUNNER
    if _RUNNER is None:
        _RUNNER = _make_runner()
    return _RUNNER


def make_in_maps(inputs, targets, centers):
    x = np.ascontiguousarray(np.asarray(inputs, dtype=np.float32))
    t = np.asarray(targets).astype(np.int64)
    c = np.ascontiguousarray(np.asarray(centers, dtype=np.float32))

    sqx = np.sum(x * x, axis=1, dtype=np.float32)          # [B]
    sqc = np.sum(c * c, axis=1, dtype=np.float32)          # [C]
    cnt = np.bincount(t, minlength=C).astype(np.float32)   # [C]
    absent = (cnt == 0).astype(np.float32)
    onehot = (t[:, None] == np.arange(C)[None, :]).astype(np.float32)  # [B, C]
    invp_row = (1.0 / cnt[t]).astype(np.float32)
    invn_row = (1.0 / (B - cnt[t])).astype(np.float32)

    xtT = np.ascontiguousarray(x.T)                        # [D, B]
    ctT = np.ascontiguousarray(c.T)                        # [D, C]
    sqcb = np.tile((sqc + BIG * absent)[None, :], (P, 1))
    pI = (2.0 * BIG) * np.eye(P, dtype=np.float32)
    cI = (1.0 - np.eye(P)).astype(BF16_NP)

    in_maps = []
    for core in range(N_CORES):
        off = core * SH
        xr = np.roll(xtT, -off, axis=1)
        ohr = np.roll(onehot, -off, axis=0)
        sqxr = np.roll(sqx, -off)
        in_maps.append({
            "xt": xr.astype(BF16_NP),
            "xst": np.ascontiguousarray(xtT[:, off:off + SH]),
            "ct": ctT,
            "ohp": np.ascontiguousarray(
                ohr.reshape(NJ, P, C).transpose(1, 0, 2).reshape(P, NJ * C)
            ).astype(BF16_NP),
            "ohs": np.ascontiguousarray(onehot[off:off + SH]),
            "ohsb": np.ascontiguousarray(BIG * onehot[off:off + SH]),
            "sqxib": np.tile(sqx[off:off + SH][None, :], (P, 1)),
            "sqxc": np.ascontiguousarray(sqxr.reshape(NJ, P).T),
            "sqcb": sqcb,
            "sqxs": np.ascontiguousarray(sqx[off:off + SH].reshape(NCH, P).T),
            "invp": np.ascontiguousarray(invp_row[off:off + SH].reshape(NCH, P).T),
            "invn": np.ascontiguousarray(invn_row[off:off + SH].reshape(NCH, P).T),
            "pI": pI,
            "cI": cI,
        })
    return in_maps


def finish(per_core_outs):
    outs = np.stack(per_core_outs).astype(np.float64)      # [8, 128, 12]
    l1 = outs[:, :, 0::3].sum()
    pr = outs[:, :, 1::3].sum()
    l2 = outs[:, :, 2::3].sum()
    loss = np.float32(l1 / B + 0.5 * (l2 / B))
    prec = np.float32(pr / B)
    return (np.asarray(loss, dtype=np.float32), np.asarray(prec, dtype=np.float32))


def kernel(inputs, targets, centers):
    in_maps = make_in_maps(inputs, targets, centers)
    results = _get_runner()(in_maps)
    return finish([results[i]["out"] for i in range(N_CORES)])
